# revision 1
# baseline (speedup 1.0000x reference)
"""Trainium2 Bass kernel (v9) for nn_AttentionBlock — reassociated causal attention.

Reference (per batch b):
    qs[t,j]    = sum_i s[t,i] Q[h,i,j]
    Omega[t,u] = sum_j qs[t,j] s[u,j]       (causal: keep u <= t)
    es[u,i]    = sum_j E[h,i,j] s[u,j]
    r[t,i]     = sum_h sum_u Omega[t,u] es[u,i]

Reassociation: for full (below-diagonal) 128-token blocks,
    sum_{u in blk} Omega[t,u] es[u,i] = qs[t,:] @ (s[blk].T @ es[blk])
so r's off-diagonal part = qs[bt] @ Gsum(bt) with Gsum the PSUM-accumulated
prefix of G_uc = s[uc].T @ es[uc]; only diagonal 128x128 Omega blocks are
materialized, masked by a DVE multiply with a precomputed triangular mask.

v3 over v2: consolidated input DMAs split across sync+scalar HWDGE queues
(cuts the serialized descriptor-issue ramp), next head's qsT/es matmul groups
software-pipelined into the current head's t-block loop (fills PE stalls on
the Gsum-snapshot chain), diag mask fused into one DVE tensor_mul (drops the
ACT-copy -> GpSimd-select chain), parallel final drain.

Distribution: data-parallel over batch (8 batches = 8 cores, no collectives).
All matmuls bf16; f32 PSUM accumulation; r lives in PSUM the whole kernel
(zeroed once, then start=False accumulate-or-overwrite via has_written).
"""

import numpy as np
import ml_dtypes

import concourse.bacc as bacc
import concourse.mybir as mybir
import concourse.tile as tile
from concourse.bass_utils import run_bass_kernel_spmd

B = 8      # batch (== number of cores)
T = 1024   # tokens
NF = 256   # feature dim n
H = 8      # heads
P = 128    # partitions
TB = T // P    # 8 token blocks
JC = NF // P   # 2 feature chunks
NCORES = 8

F32 = mybir.dt.float32
BF16 = mybir.dt.bfloat16
IS_GE = mybir.AluOpType.is_ge


def _emit(tc, nc, s_d, sT_d, Q_d, ET_d, out_d, ctx):
    res = ctx.enter_context(tc.tile_pool(name="res", bufs=1))
    work = ctx.enter_context(tc.tile_pool(name="work", bufs=2))
    snap = ctx.enter_context(tc.tile_pool(name="snap", bufs=3))
    prp = ctx.enter_context(tc.tile_pool(name="prp", bufs=1, space="PSUM"))
    pgp = ctx.enter_context(tc.tile_pool(name="pgp", bufs=1, space="PSUM"))
    pwp = ctx.enter_context(tc.tile_pool(name="pwp", bufs=3, space="PSUM"))

    s_sb = res.tile([P, TB, NF], BF16)      # [u%128, uc, j]
    sT_sb = res.tile([P, JC, T], BF16)      # [j%128, jc, t]
    Q_sb = res.tile([P, H * JC, NF], BF16)  # [i%128, h*2+ic, j]
    ET_sb = res.tile([P, H * JC, NF], BF16)  # [j%128, h*2+jc, i]
    mask = res.tile([P, 2, P], BF16)        # [u, pair, t]: 1 where u <= t
    r_out = res.tile([P, TB, NF], F32)

    # r accumulates here across the whole kernel: zero once, then every
    # matmul uses start=False (accumulate-or-overwrite via has_written).
    pr = prp.tile([P, TB, NF], F32)
    nc.vector.memset(pr, 0.0)

    nc.gpsimd.memset(mask, 1.0)
    nc.gpsimd.affine_select(
        out=mask, in_=mask,
        pattern=[[0, 2], [1, P]],
        compare_op=IS_GE,   # keep 1.0 where t - u >= 0, else 0
        fill=0.0, base=0, channel_multiplier=-1,
    )

    # Input DMAs: few big transfers, first-needed first, split across the
    # sync and scalar HWDGE queues so descriptor issue isn't serialized.
    nc.sync.dma_start(
        out=Q_sb[:, 0:JC, :],
        in_=Q_d[0].rearrange("(c p) j -> p c j", p=P))
    nc.sync.dma_start(
        out=sT_sb[:, :, 0:512],
        in_=sT_d[:, 0:512].rearrange("(c p) t -> p c t", p=P))
    nc.sync.dma_start(
        out=sT_sb[:, :, 512:],
        in_=sT_d[:, 512:].rearrange("(c p) t -> p c t", p=P))
    nc.sync.dma_start(
        out=Q_sb[:, JC:2 * JC, :],
        in_=Q_d[1].rearrange("(c p) j -> p c j", p=P))
    nc.sync.dma_start(
        out=s_sb, in_=s_d.rearrange("(c p) j -> p c j", p=P))
    nc.sync.dma_start(
        out=Q_sb[:, 2 * JC:4 * JC, :],
        in_=Q_d[2:4].rearrange("h (c p) j -> p (h c) j", p=P))
    nc.sync.dma_start(
        out=Q_sb[:, 4 * JC:, :],
        in_=Q_d[4:].rearrange("h (c p) j -> p (h c) j", p=P))
    nc.gpsimd.dma_start(
        out=ET_sb[:, 0:JC, :],
        in_=ET_d[0].rearrange("(c p) j -> p c j", p=P))
    nc.gpsimd.dma_start(
        out=ET_sb[:, JC:4 * JC, :],
        in_=ET_d[1:4].rearrange("h (c p) j -> p (h c) j", p=P))
    nc.gpsimd.dma_start(
        out=ET_sb[:, 4 * JC:, :],
        in_=ET_d[4:].rearrange("h (c p) j -> p (h c) j", p=P))

    movers = [nc.vector.tensor_copy, nc.scalar.copy]
    mv = [0]

    def mover(out, in_):
        movers[mv[0] % 2](out=out, in_=in_)
        mv[0] += 1

    # ---- per-head prep: qsT and es, emitted as 8 independent PSUM groups
    def prep_groups(h):
        """Yields 8 emit-thunks: 4 qsT groups then 4 es pair-groups."""
        qsT = work.tile([P, JC, T], BF16, tag="qsT", name=f"qsT{h}")
        es = work.tile([P, TB, NF], BF16, tag="es", name=f"es{h}")

        def qsT_group(jc, tcx):
            pw = pwp.tile([P, 512], F32, tag="pw", name="pwq")
            for ic in range(JC):
                nc.tensor.matmul(
                    pw,
                    lhsT=Q_sb[:, h * JC + ic, jc * P:(jc + 1) * P],
                    rhs=sT_sb[:, ic, tcx * 512:(tcx + 1) * 512],
                    start=(ic == 0),
                    stop=(ic == JC - 1),
                )
            mover(qsT[:, jc, tcx * 512:(tcx + 1) * 512], pw)

        def es_group(up):
            pw = pwp.tile([P, 512], F32, tag="pw", name="pwe")
            for half in range(2):
                uc = 2 * up + half
                for jc in range(JC):
                    nc.tensor.matmul(
                        pw[:, half * NF:(half + 1) * NF],
                        lhsT=sT_sb[:, jc, uc * P:(uc + 1) * P],
                        rhs=ET_sb[:, h * JC + jc, :],
                        start=(half == 0 and jc == 0),
                        stop=(half == 1 and jc == JC - 1),
                        skip_group_check=True,
                    )
            mover(es[:, 2 * up:2 * up + 2, :], pw)

        thunks = []
        for jc in range(JC):
            for tcx in range(T // 512):
                thunks.append(lambda jc=jc, tcx=tcx: qsT_group(jc, tcx))
        for up in range(TB // 2):
            thunks.append(lambda up=up: es_group(up))
        return qsT, es, thunks

    def body(h, qsT, es, next_thunks, drain=False):
        """bt-loop for head h, with next head's prep groups interleaved."""
        pg = pgp.tile([P, JC, NF], F32, tag="pg")
        pwd = None
        omd = None
        for bt in range(TB):
            # diagonal OmegaT block [u,t] = sum_j sT[j,u] qsT[j,t]; two bt
            # share a bank; masked via one DVE multiply (keep u <= t)
            if bt % 2 == 0:
                pwd = pwp.tile([P, 512], F32, tag="pw", name="pwd")
                omd = snap.tile([P, 2, P], BF16, tag="omd")
            for jc in range(JC):
                nc.tensor.matmul(
                    pwd[:, (bt % 2) * P:(bt % 2 + 1) * P],
                    lhsT=sT_sb[:, jc, bt * P:(bt + 1) * P],
                    rhs=qsT[:, jc, bt * P:(bt + 1) * P],
                    start=(bt % 2 == 0 and jc == 0),
                    stop=(bt % 2 == 1 and jc == JC - 1),
                    skip_group_check=True,
                )
            # full part first: Gsum(bt) = sum_{uc<bt} s[uc].T @ es[uc]
            # (PSUM prefix accumulation); its PE matmuls run while the DVE
            # mask-multiply of the diag block completes
            gs = None
            if bt >= 1:
                uc = bt - 1
                for jc in range(JC):
                    nc.tensor.matmul(
                        pg[:, jc, :],
                        lhsT=s_sb[:, uc, jc * P:(jc + 1) * P],
                        rhs=es[:, uc, :],
                        start=(bt == 1 and jc == 0),
                        stop=(bt == TB - 1 and jc == JC - 1),
                        skip_group_check=True,
                    )
                if drain:
                    # bare chain: halve snapshot latency via both engines
                    gs = snap.tile([P, JC, NF], BF16, tag="gs")
                    nc.vector.tensor_copy(out=gs[:, 0, :], in_=pg[:, 0, :])
                    nc.scalar.copy(out=gs[:, 1, :], in_=pg[:, 1, :])
                else:
                    gs = snap.tile([P, JC, NF], BF16, tag="gs")
                    mover(gs, pg)
            if next_thunks:
                next_thunks.pop(0)()
            if bt % 2 == 1:
                nc.vector.tensor_mul(omd, pwd[:, 0:2 * P], mask)
                for b2 in (bt - 1, bt):
                    nc.tensor.matmul(
                        pr[:, b2, :],
                        lhsT=omd[:, b2 % 2, :],
                        rhs=es[:, b2, :],
                        start=False, stop=False, skip_group_check=True,
                    )
            if gs is not None:
                for jc in range(JC):
                    nc.tensor.matmul(
                        pr[:, bt, :],
                        lhsT=qsT[:, jc, bt * P:(bt + 1) * P],
                        rhs=gs[:, jc, :],
                        start=False, stop=False, skip_group_check=True,
                    )
            if drain and bt % 2 == 1:
                # h7: pr[bt-1], pr[bt] are final after this iteration
                mover(r_out[:, bt - 1:bt + 1, :], pr[:, bt - 1:bt + 1, :])
                if bt == 3:
                    nc.sync.dma_start(
                        out=out_d[0:T // 2, :].rearrange(
                            "(bt p) i -> p bt i", p=P),
                        in_=r_out[:, 0:TB // 2, :])
                elif bt == 5:
                    nc.scalar.dma_start(
                        out=out_d[T // 2:3 * T // 4, :].rearrange(
                            "(bt p) i -> p bt i", p=P),
                        in_=r_out[:, 4:6, :])
                elif bt == 7:
                    nc.sync.dma_start(
                        out=out_d[3 * T // 4:, :].rearrange(
                            "(bt p) i -> p bt i", p=P),
                        in_=r_out[:, 6:8, :])

    qsT, es, thunks = prep_groups(0)
    for th in thunks:
        th()
    for h in range(H):
        if h + 1 < H:
            nqsT, nes, nthunks = prep_groups(h + 1)
        else:
            nqsT, nes, nthunks = None, None, []
        body(h, qsT, es, nthunks, drain=(h == H - 1))
        for th in nthunks:   # any leftovers
            th()
        qsT, es = nqsT, nes



def build():
    from contextlib import ExitStack

    nc = bacc.Bacc(
        "TRN2",
        target_bir_lowering=False,
        debug=False,
        enable_asserts=False,
        num_devices=NCORES,
    )
    s_d = nc.dram_tensor("s", [T, NF], BF16, kind="ExternalInput").ap()
    sT_d = nc.dram_tensor("sT", [NF, T], BF16, kind="ExternalInput").ap()
    Q_d = nc.dram_tensor("Q", [H, NF, NF], BF16, kind="ExternalInput").ap()
    ET_d = nc.dram_tensor("ET", [H, NF, NF], BF16, kind="ExternalInput").ap()
    out_d = nc.dram_tensor("out", [T, NF], F32, kind="ExternalOutput").ap()
    with tile.TileContext(nc) as tc:
        with ExitStack() as ctx:
            _emit(tc, nc, s_d, sT_d, Q_d, ET_d, out_d, ctx)
    nc.compile()
    return nc


_NC = None


def _get_nc():
    global _NC
    if _NC is None:
        _NC = build()
    return _NC


def _in_maps(s, Q, E):
    bf = ml_dtypes.bfloat16
    s = np.asarray(s, dtype=np.float32)
    Qb = np.ascontiguousarray(np.asarray(Q, dtype=np.float32)).astype(bf)
    ETb = np.ascontiguousarray(
        np.asarray(E, dtype=np.float32).transpose(0, 2, 1)).astype(bf)
    return [
        {
            "s": np.ascontiguousarray(s[b]).astype(bf),
            "sT": np.ascontiguousarray(s[b].T).astype(bf),
            "Q": Qb,
            "ET": ETb,
        }
        for b in range(B)
    ]


def kernel(s, Q, E):
    nc = _get_nc()
    res = run_bass_kernel_spmd(
        nc, _in_maps(s, Q, E), core_ids=list(range(NCORES)))
    return np.stack([res.results[b]["out"] for b in range(B)], axis=0)


def run_profiled(s, Q, E, tmpdir=None):
    nc = _get_nc()
    res = run_bass_kernel_spmd(
        nc, _in_maps(s, Q, E), core_ids=list(range(NCORES)),
        trace=True, tmpdir=tmpdir)
    out = np.stack([res.results[b]["out"] for b in range(B)], axis=0)
    return out, res.exec_time_ns



# revision 21
# speedup vs baseline: 1.0637x; 1.0637x over previous
"""Trainium2 Bass kernel (v10) for nn_AttentionBlock — reassociated causal attention.

Reference (per batch b):
    qs[t,j]    = sum_i s[t,i] Q[h,i,j]
    Omega[t,u] = sum_j qs[t,j] s[u,j]       (causal: keep u <= t)
    es[u,i]    = sum_j E[h,i,j] s[u,j]
    r[t,i]     = sum_h sum_u Omega[t,u] es[u,i]

Reassociation: for full (below-diagonal) 128-token blocks,
    sum_{u in blk} Omega[t,u] es[u,i] = qs[t,:] @ (s[blk].T @ es[blk])
so r's off-diagonal part = qs[bt] @ Gsum(bt) with Gsum the PSUM-accumulated
prefix of G_uc = s[uc].T @ es[uc]; only diagonal 128x128 Omega blocks are
materialized, masked by a DVE multiply with a precomputed triangular mask.

v10 over v9: heads processed in PAIRS with concatenated rhs operands (es, G,
and diag-Omega matmuls run at N=512/N=256 over both heads — ~30% fewer PE
instructions), r accumulated in SBUF f32 via per-chunk DVE adds (frees 4 PSUM
banks for deeper rotation and removes the end-of-kernel PSUM drain), the
Gsum snapshot is consumed one bt-iteration late (software pipeline — PE never
waits on the PSUM->SBUF cast), all DRAM tensors are pre-swizzled on the host
into [128, contiguous] layouts (few large full-bandwidth DMAs, ~2KB+ lines),
and a short burst of dummy matmuls at t=0 warms the PE HAM clock gate during
the input-DMA ramp.

Distribution: data-parallel over batch (8 batches = 8 cores, no collectives).
All matmuls bf16; f32 PSUM accumulation.
"""

import numpy as np
import ml_dtypes

import concourse.bacc as bacc
import concourse.mybir as mybir
import concourse.tile as tile
from concourse.bass_utils import run_bass_kernel_spmd

B = 8      # batch (== number of cores)
T = 1024   # tokens
NF = 256   # feature dim n
H = 8      # heads
P = 128    # partitions
TB = T // P    # 8 token blocks
JC = NF // P   # 2 feature chunks
NPAIR = H // 2
NCORES = 8

F32 = mybir.dt.float32
BF16 = mybir.dt.bfloat16
IS_GE = mybir.AluOpType.is_ge


def _emit(tc, nc, s_d, sT_d, Q_d, ET_d, out_d, ctx):
    res = ctx.enter_context(tc.tile_pool(name="res", bufs=1))
    work = ctx.enter_context(tc.tile_pool(name="work", bufs=2))
    snap = ctx.enter_context(tc.tile_pool(name="snap", bufs=2))
    prp = ctx.enter_context(tc.tile_pool(name="prp", bufs=2, space="PSUM"))
    pgp = ctx.enter_context(tc.tile_pool(name="pgp", bufs=1, space="PSUM"))
    pwp = ctx.enter_context(tc.tile_pool(name="pwp", bufs=2, space="PSUM"))
    pdp = ctx.enter_context(tc.tile_pool(name="pdp", bufs=2, space="PSUM"))

    s_sb = res.tile([P, TB, NF], BF16)       # [u%128, uc, j]
    sT_sb = res.tile([P, 2, JC, 512], BF16)  # [j%128, tcx, jc, t']
    Q_sb = res.tile([P, H * JC, NF], BF16)   # [i%128, h*2+ic, j]
    ET_sb = res.tile([P, JC * H, NF], BF16)  # [j%128, jc*8+h, i]
    mask4 = res.tile([P, 4, P], BF16)        # [u, (bt%2, h), t]: 1 where u <= t
    warm = res.tile([P, 640], BF16)
    r_sb = res.tile([P, TB, NF], F32)        # final r accumulator (SBUF)

    # ---- input DMAs: [128, contiguous] slices, first-needed first, spread
    # across the sync/scalar/gpsimd HWDGE queues (~1MB each).
    nc.sync.dma_start(out=sT_sb[:, 0], in_=sT_d[:, 0])
    nc.scalar.dma_start(out=Q_sb[:, 0:4, :], in_=Q_d[:, 0:4, :])
    nc.gpsimd.dma_start(out=ET_sb[:, 0:2, :], in_=ET_d[:, 0:2, :])
    nc.gpsimd.dma_start(out=ET_sb[:, 8:10, :], in_=ET_d[:, 8:10, :])
    nc.sync.dma_start(out=sT_sb[:, 1], in_=sT_d[:, 1])
    nc.sync.dma_start(out=s_sb, in_=s_d)
    nc.scalar.dma_start(out=Q_sb[:, 4:8, :], in_=Q_d[:, 4:8, :])
    nc.gpsimd.dma_start(out=ET_sb[:, 2:4, :], in_=ET_d[:, 2:4, :])
    nc.gpsimd.dma_start(out=ET_sb[:, 10:12, :], in_=ET_d[:, 10:12, :])
    nc.scalar.dma_start(out=Q_sb[:, 8:16, :], in_=Q_d[:, 8:16, :])
    nc.gpsimd.dma_start(out=ET_sb[:, 4:8, :], in_=ET_d[:, 4:8, :])
    nc.gpsimd.dma_start(out=ET_sb[:, 12:16, :], in_=ET_d[:, 12:16, :])

    # warm tile zeroed on the (idle) vector engine so warmup matmuls are not
    # queued behind gpsimd's DMA issues; mask setup stays on gpsimd.
    nc.vector.memset(warm, 0.0)
    nc.gpsimd.memset(mask4, 1.0)
    nc.gpsimd.affine_select(
        out=mask4, in_=mask4,
        pattern=[[0, 4], [1, P]],
        compare_op=IS_GE,   # keep 1.0 where t - u >= 0, else 0
        fill=0.0, base=0, channel_multiplier=-1,
    )

    # HAM warmup: dummy matmuls on a zeroed tile while input DMAs run, so the
    # PE clock gate is at 8/8 by the time real matmuls start.
    pwarm = pdp.tile([P, 512], F32, tag="pwd", name="pwarm")
    for _ in range(6):
        nc.tensor.matmul(pwarm, lhsT=warm[:, 0:128], rhs=warm[:, 128:640],
                         start=True, stop=True, skip_group_check=True)

    # In-body prep evacuations go to ScalarE (VectorE carries the gs
    # copies, masks, and r-chunk adds); the bare prologue and the last
    # pair's deferred prep alternate both engines.
    movers = [nc.vector.tensor_copy, nc.scalar.copy]
    mv = [0]

    def mover(out, in_, alt=False):
        if alt:
            movers[mv[0] % 2](out=out, in_=in_)
            mv[0] += 1
        else:
            nc.scalar.copy(out=out, in_=in_)

    # ---- per-pair prep: qsT for both heads and pair-concatenated es
    def prep_groups(p, pool_cycle=None, alt_from=None):
        h0 = 2 * p
        qsT2 = work.tile([P, 2, JC, T], BF16, tag="qsT", name=f"qsT{p}")
        es2 = work.tile([P, TB, 2 * NF], BF16, tag="es", name=f"es{p}")
        base_alt = pool_cycle is not None
        pool_cycle = pool_cycle or [(pwp, "pw")]

        def qsT_group(hh, jc, tcx, pool, tag, alt):
            pw = pool.tile([P, 512], F32, tag=tag, name="pwq")
            for ic in range(JC):
                nc.tensor.matmul(
                    pw,
                    lhsT=Q_sb[:, (h0 + hh) * 2 + ic, jc * P:(jc + 1) * P],
                    rhs=sT_sb[:, tcx, ic, :],
                    start=(ic == 0), stop=(ic == JC - 1),
                    skip_group_check=True,
                )
            mover(qsT2[:, hh, jc, tcx * 512:(tcx + 1) * 512], pw, alt=alt)

        def es_group(uc, pool, tag, alt):
            pw = pool.tile([P, 512], F32, tag=tag, name="pwe")
            for jc in range(JC):
                nc.tensor.matmul(
                    pw,
                    lhsT=sT_sb[:, uc // 4, jc, (uc % 4) * P:(uc % 4 + 1) * P],
                    rhs=ET_sb[:, jc * H + h0:jc * H + h0 + 2, :],
                    start=(jc == 0), stop=(jc == JC - 1),
                    skip_group_check=True,
                )
            mover(es2[:, uc, :], pw, alt=alt)

        # Order groups progressively: tcx0-qsT and low-uc es first (their
        # DMAs land first in the prologue); the tail of the list is safe to
        # defer into the consuming pair's own body (late-uc es / tcx1 qsT).
        specs = []
        for hh in range(2):
            for jc in range(JC):
                specs.append(("q", (hh, jc, 0)))
        for uc in range(3):
            specs.append(("e", (uc,)))
        for hh in range(2):
            for jc in range(JC):
                specs.append(("q", (hh, jc, 1)))
        for uc in range(3, TB):
            specs.append(("e", (uc,)))
        thunks = []
        for i, (kind, args) in enumerate(specs):
            pool, tag = pool_cycle[i % len(pool_cycle)]
            alt = base_alt or (alt_from is not None and i >= alt_from)
            if kind == "q":
                thunks.append(
                    lambda a=args, pool=pool, tag=tag, alt=alt:
                    qsT_group(*a, pool, tag, alt))
            else:
                thunks.append(
                    lambda a=args, pool=pool, tag=tag, alt=alt:
                    es_group(*a, pool, tag, alt))
        return qsT2, es2, thunks

    def body(p, qsT2, es2, nthunks, drain, drate=2):
        pg2 = pgp.tile([P, JC, 512], F32, tag="pg", name=f"pg{p}")
        gs_prev = None
        gs = None
        pwd = None
        omd = None
        rp = [None] * 4        # rp chunk tiles, one per 2-bt
        rp_started = [False] * 4
        for bt in range(TB + 1):
            if bt < TB:
                # [A] diag OmegaT block for both heads: [u, (h, t)]
                if bt % 2 == 0:
                    pwd = pdp.tile([P, 4, P], F32, tag="pwd", name="pwd")
                for jc in range(JC):
                    nc.tensor.matmul(
                        pwd[:, 2 * (bt % 2):2 * (bt % 2) + 2, :],
                        lhsT=sT_sb[:, bt // 4, jc, (bt % 4) * P:(bt % 4 + 1) * P],
                        rhs=qsT2[:, :, jc, bt * P:(bt + 1) * P],
                        start=(bt % 2 == 0 and jc == 0),
                        stop=(bt % 2 == 1 and jc == JC - 1),
                        skip_group_check=True,
                    )
            # [G] r_off(bt-1) = qs[bt-1] @ Gsum[bt-1] via last iter's snapshot.
            # MUST be emitted before [B]: [G]'s wait on the gs copy transitively
            # (via the PE FIFO) keeps this iteration's pg2-accumulating matmuls
            # from racing ahead of last iteration's snapshot read.
            if bt >= 2:
                tb = bt - 1
                k = tb // 2
                if rp[k] is None:
                    rp[k] = prp.tile([P, 2, NF], F32, tag="rp", name=f"rp{k}")
                for hh in range(2):
                    for jc in range(JC):
                        nc.tensor.matmul(
                            rp[k][:, tb % 2, :],
                            lhsT=qsT2[:, hh, jc, tb * P:(tb + 1) * P],
                            rhs=gs_prev[:, jc, hh * NF:(hh + 1) * NF],
                            start=(not rp_started[k]),
                            stop=(tb % 2 == 1 and hh == 1 and jc == JC - 1),
                            skip_group_check=True,
                        )
                        rp_started[k] = True
            if bt < TB:
                # [B] Gsum prefix accumulation + [C] snapshot (consumed next iter)
                if bt >= 1:
                    uc = bt - 1
                    for jc in range(JC):
                        nc.tensor.matmul(
                            pg2[:, jc, :],
                            lhsT=s_sb[:, uc, jc * P:(jc + 1) * P],
                            rhs=es2[:, uc, :],
                            # pg2 spans two banks (one per jc): each bank's
                            # first matmul needs start=True to clear its own
                            # has_written bits (start only clears ONE bank).
                            start=(bt == 1),
                            stop=(bt == TB - 1 and jc == JC - 1),
                            skip_group_check=True,
                        )
                    gs = snap.tile([P, JC, 512], BF16, tag="gs")
                    nc.vector.tensor_copy(out=gs[:, 0], in_=pg2[:, 0])
                    nc.scalar.copy(out=gs[:, 1], in_=pg2[:, 1])
            # [H] chunk complete -> accumulate into SBUF r, drain if last.
            # Placed after [C] so the gs copy leads the DVE queue.
            if bt >= 2 and (bt - 1) % 2 == 1:
                k = (bt - 1) // 2
                sl = r_sb[:, 2 * k:2 * k + 2, :]
                if p == 0:
                    nc.vector.tensor_copy(out=sl, in_=rp[k])
                else:
                    nc.vector.tensor_add(out=sl, in0=rp[k], in1=sl)
                if drain:
                    nc.sync.dma_start(out=out_d[:, 2 * k:2 * k + 2, :], in_=sl)
            if bt < TB:
                # [D] prep groups, interleaved as PE filler
                for _ in range(drate):
                    if nthunks:
                        nthunks.pop(0)()
                # [E]+[F] mask the diag pair, then its r contribution
                if bt % 2 == 1:
                    omd = snap.tile([P, 4, P], BF16, tag="omd")
                    nc.vector.tensor_mul(omd, pwd, mask4)
                    for b2 in (bt - 1, bt):
                        k = b2 // 2
                        if rp[k] is None:
                            rp[k] = prp.tile([P, 2, NF], F32, tag="rp",
                                             name=f"rp{k}")
                        for hh in range(2):
                            nc.tensor.matmul(
                                rp[k][:, b2 % 2, :],
                                lhsT=omd[:, 2 * (b2 % 2) + hh, :],
                                rhs=es2[:, b2, hh * NF:(hh + 1) * NF],
                                start=(not rp_started[k]),
                                stop=False,
                                skip_group_check=True,
                            )
                            rp_started[k] = True
            gs_prev = gs

    # pair-0 prep runs bare during the DMA ramp; rotate over all three
    # transient PSUM pools so evacuation latency never blocks the PE. The
    # deferred bulk DMAs are emitted between groups so the scalar/gpsimd
    # queues enqueue them only once the critical transfers are in flight.
    qsT2, es2, thunks = prep_groups(
        0, pool_cycle=[(pwp, "pw"), (pdp, "pwd"), (prp, "rp")])
    for th in thunks:
        th()
    carry = []   # pair-3 prep groups deferred into pair 3's own body as
    # PE filler (it has no next-pair prep to hide the gs-copy latency behind)
    for p in range(NPAIR):
        if p + 1 < NPAIR:
            nqsT2, nes2, nthunks = prep_groups(
                p + 1, alt_from=(7 if p + 1 == NPAIR - 1 else None))
            if p + 1 == NPAIR - 1:
                nthunks, carry = nthunks[:7], nthunks[7:]
        else:
            nqsT2, nes2, nthunks = None, None, carry
        body(p, qsT2, es2, nthunks, drain=(p == NPAIR - 1),
             drate=(1 if p == NPAIR - 2 else 2))
        for th in nthunks:   # any leftovers
            th()
        qsT2, es2 = nqsT2, nes2


def build():
    from contextlib import ExitStack

    nc = bacc.Bacc(
        "TRN2",
        target_bir_lowering=False,
        debug=False,
        enable_asserts=False,
        num_devices=NCORES,
    )
    s_d = nc.dram_tensor("s", [P, TB, NF], BF16, kind="ExternalInput").ap()
    sT_d = nc.dram_tensor("sT", [P, 2, JC, 512], BF16,
                          kind="ExternalInput").ap()
    Q_d = nc.dram_tensor("Q", [P, H * JC, NF], BF16, kind="ExternalInput").ap()
    ET_d = nc.dram_tensor("ET", [P, JC * H, NF], BF16,
                          kind="ExternalInput").ap()
    out_d = nc.dram_tensor("out", [P, TB, NF], F32, kind="ExternalOutput").ap()
    with tile.TileContext(nc) as tc:
        with ExitStack() as ctx:
            _emit(tc, nc, s_d, sT_d, Q_d, ET_d, out_d, ctx)
    nc.compile()
    return nc


_NC = None


def _get_nc():
    global _NC
    if _NC is None:
        _NC = build()
    return _NC


def _in_maps(s, Q, E):
    bf = ml_dtypes.bfloat16
    s = np.asarray(s, np.float32)
    Qf = np.asarray(Q, np.float32)
    Ef = np.asarray(E, np.float32)
    Qd = np.ascontiguousarray(
        Qf.reshape(H, JC, P, NF).transpose(2, 0, 1, 3).reshape(P, H * JC, NF)
    ).astype(bf)
    ETd = np.ascontiguousarray(
        Ef.transpose(2, 0, 1).reshape(JC, P, H, NF).transpose(1, 0, 2, 3)
        .reshape(P, JC * H, NF)
    ).astype(bf)
    maps = []
    for b in range(B):
        sb = s[b]
        sd = np.ascontiguousarray(
            sb.reshape(TB, P, NF).transpose(1, 0, 2)).astype(bf)
        sTd = np.ascontiguousarray(
            sb.T.reshape(JC, P, 2, 512).transpose(1, 2, 0, 3)).astype(bf)
        maps.append({"s": sd, "sT": sTd, "Q": Qd, "ET": ETd})
    return maps


def _unpack(res):
    return np.stack([
        np.ascontiguousarray(
            res.results[b]["out"].transpose(1, 0, 2).reshape(T, NF))
        for b in range(B)], axis=0)


def kernel(s, Q, E):
    nc = _get_nc()
    res = run_bass_kernel_spmd(
        nc, _in_maps(s, Q, E), core_ids=list(range(NCORES)))
    return _unpack(res)


def run_profiled(s, Q, E, tmpdir=None):
    nc = _get_nc()
    res = run_bass_kernel_spmd(
        nc, _in_maps(s, Q, E), core_ids=list(range(NCORES)),
        trace=True, tmpdir=tmpdir)
    return _unpack(res), res.exec_time_ns


# revision 25
# speedup vs baseline: 1.1319x; 1.0641x over previous
"""Trainium2 Bass kernel (v10) for nn_AttentionBlock — reassociated causal attention.

Reference (per batch b):
    qs[t,j]    = sum_i s[t,i] Q[h,i,j]
    Omega[t,u] = sum_j qs[t,j] s[u,j]       (causal: keep u <= t)
    es[u,i]    = sum_j E[h,i,j] s[u,j]
    r[t,i]     = sum_h sum_u Omega[t,u] es[u,i]

Reassociation: for full (below-diagonal) 128-token blocks,
    sum_{u in blk} Omega[t,u] es[u,i] = qs[t,:] @ (s[blk].T @ es[blk])
so r's off-diagonal part = qs[bt] @ Gsum(bt) with Gsum the PSUM-accumulated
prefix of G_uc = s[uc].T @ es[uc]; only diagonal 128x128 Omega blocks are
materialized, masked by a DVE multiply with a precomputed triangular mask.

v13 over v9: heads processed in PAIRS with concatenated rhs operands (es, G,
and diag-Omega matmuls run at N=512/N=256 over both heads — 422 vs 601 PE
instructions), r accumulated in SBUF f32 via per-chunk DVE adds (frees 4 PSUM
banks for deeper transient rotation and removes the end-of-kernel PSUM
drain), the Gsum snapshot is consumed one bt-iteration late (software
pipeline; the r_off matmuls are emitted BEFORE the G-accumulate so the PE
FIFO transitively orders each snapshot read before the next accumulate —
Tile does not emit that WAR edge itself), all DRAM tensors are pre-swizzled
on the host into [128, contiguous] layouts (12 large 2KB+/partition-line
DMAs at full HBM bandwidth), a burst of dummy matmuls at t=0 warms the PE
HAM clock gate during the input-DMA ramp, and the last head-pair (which has
no next-pair prep) gets 9 of its own prep groups deferred into its body as
PE filler.

Gotcha encoded here: a PSUM accumulation group spanning N banks needs
start=True on EACH bank's first matmul (start clears has_written for one
bank only).

Distribution: data-parallel over batch (8 batches = 8 cores, no collectives).
All matmuls bf16; f32 PSUM accumulation. ~106us vs 111us baseline; rel err
3.09e-3 (identical to baseline). Note: back-to-back benchmarking runs heat
the part into a lower power state (~+20%); space measurement runs out.
"""

import numpy as np
import ml_dtypes

import concourse.bacc as bacc
import concourse.mybir as mybir
import concourse.tile as tile
from concourse.bass_utils import run_bass_kernel_spmd

B = 8      # batch (== number of cores)
T = 1024   # tokens
NF = 256   # feature dim n
H = 8      # heads
P = 128    # partitions
TB = T // P    # 8 token blocks
JC = NF // P   # 2 feature chunks
NPAIR = H // 2
NCORES = 8

F32 = mybir.dt.float32
BF16 = mybir.dt.bfloat16
IS_GE = mybir.AluOpType.is_ge


def _emit(tc, nc, s_d, sT_d, Q_d, ET_d, out_d, ctx):
    res = ctx.enter_context(tc.tile_pool(name="res", bufs=1))
    work = ctx.enter_context(tc.tile_pool(name="work", bufs=2))
    snap = ctx.enter_context(tc.tile_pool(name="snap", bufs=2))
    prp = ctx.enter_context(tc.tile_pool(name="prp", bufs=1, space="PSUM"))
    pgp = ctx.enter_context(tc.tile_pool(name="pgp", bufs=1, space="PSUM"))
    pwp = ctx.enter_context(tc.tile_pool(name="pwp", bufs=2, space="PSUM"))
    pdp = ctx.enter_context(tc.tile_pool(name="pdp", bufs=3, space="PSUM"))

    s_sb = res.tile([P, TB, NF], BF16)       # [u%128, uc, j]
    sT_sb = res.tile([P, 2, JC, 512], BF16)  # [j%128, tcx, jc, t']
    Q_sb = res.tile([P, H * JC, NF], BF16)   # [i%128, h*2+ic, j]
    ET_sb = res.tile([P, JC * H, NF], BF16)  # [j%128, jc*8+h, i]
    mask4 = res.tile([P, 4, P], BF16)        # [u, (bt%2, h), t]: 1 where u <= t
    warm = res.tile([P, 640], BF16)
    r_sb = res.tile([P, TB, NF], F32)        # final r accumulator (SBUF)

    # ---- input DMAs: [128, contiguous] slices. The critical first wave
    # (both sT halves, pair-0 Q and ET) rides four queues in parallel so the
    # whole prologue working set lands ~3us after issue; bulk follows.
    nc.sync.dma_start(out=sT_sb[:, 0], in_=sT_d[:, 0])
    nc.sync.dma_start(out=sT_sb[:, 1], in_=sT_d[:, 1])
    nc.scalar.dma_start(out=Q_sb[:, 0:4, :], in_=Q_d[:, 0:4, :])
    nc.gpsimd.dma_start(out=ET_sb[:, 0:2, :], in_=ET_d[:, 0:2, :])
    nc.gpsimd.dma_start(out=ET_sb[:, 8:10, :], in_=ET_d[:, 8:10, :])
    nc.sync.dma_start(out=s_sb, in_=s_d)
    nc.scalar.dma_start(out=Q_sb[:, 4:8, :], in_=Q_d[:, 4:8, :])
    nc.gpsimd.dma_start(out=ET_sb[:, 2:4, :], in_=ET_d[:, 2:4, :])
    nc.gpsimd.dma_start(out=ET_sb[:, 10:12, :], in_=ET_d[:, 10:12, :])
    nc.scalar.dma_start(out=Q_sb[:, 8:16, :], in_=Q_d[:, 8:16, :])
    nc.gpsimd.dma_start(out=ET_sb[:, 4:8, :], in_=ET_d[:, 4:8, :])
    nc.gpsimd.dma_start(out=ET_sb[:, 12:16, :], in_=ET_d[:, 12:16, :])

    # warm tile zeroed on the (idle) vector engine so warmup matmuls are not
    # queued behind gpsimd's DMA issues; mask setup stays on gpsimd.
    nc.vector.memset(warm, 0.0)
    nc.gpsimd.memset(mask4, 1.0)
    nc.gpsimd.affine_select(
        out=mask4, in_=mask4,
        pattern=[[0, 4], [1, P]],
        compare_op=IS_GE,   # keep 1.0 where t - u >= 0, else 0
        fill=0.0, base=0, channel_multiplier=-1,
    )

    # HAM warmup: dummy matmuls on a zeroed tile while input DMAs run, so the
    # PE clock gate is at 8/8 by the time real matmuls start.
    pwarm = pdp.tile([P, 512], F32, tag="pwd", name="pwarm")
    for _ in range(9):
        nc.tensor.matmul(pwarm, lhsT=warm[:, 0:128], rhs=warm[:, 128:640],
                         start=True, stop=True, skip_group_check=True)

    # In-body prep evacuations go to ScalarE (VectorE carries the gs
    # copies, masks, and r-chunk adds); the bare prologue and the last
    # pair's deferred prep alternate both engines.
    movers = [nc.vector.tensor_copy, nc.scalar.copy]
    mv = [0]

    def mover(out, in_, alt=False):
        if alt:
            movers[mv[0] % 2](out=out, in_=in_)
            mv[0] += 1
        else:
            nc.scalar.copy(out=out, in_=in_)

    # ---- per-pair prep: qsT for both heads and pair-concatenated es
    def prep_groups(p, pool_cycle=None, alt_from=None):
        h0 = 2 * p
        qsT2 = work.tile([P, 2, JC, T], BF16, tag="qsT", name=f"qsT{p}")
        es2 = work.tile([P, TB, 2 * NF], BF16, tag="es", name=f"es{p}")
        base_alt = pool_cycle is not None
        pool_cycle = pool_cycle or [(pwp, "pw")]

        def qsT_group(hh, jc, tcx, pool, tag, alt):
            pw = pool.tile([P, 512], F32, tag=tag, name="pwq")
            for ic in range(JC):
                nc.tensor.matmul(
                    pw,
                    lhsT=Q_sb[:, (h0 + hh) * 2 + ic, jc * P:(jc + 1) * P],
                    rhs=sT_sb[:, tcx, ic, :],
                    start=(ic == 0), stop=(ic == JC - 1),
                    skip_group_check=True,
                )
            mover(qsT2[:, hh, jc, tcx * 512:(tcx + 1) * 512], pw, alt=alt)

        def es_group(uc, pool, tag, alt):
            pw = pool.tile([P, 512], F32, tag=tag, name="pwe")
            for jc in range(JC):
                nc.tensor.matmul(
                    pw,
                    lhsT=sT_sb[:, uc // 4, jc, (uc % 4) * P:(uc % 4 + 1) * P],
                    rhs=ET_sb[:, jc * H + h0:jc * H + h0 + 2, :],
                    start=(jc == 0), stop=(jc == JC - 1),
                    skip_group_check=True,
                )
            mover(es2[:, uc, :], pw, alt=alt)

        # Order groups progressively: tcx0-qsT and low-uc es first (their
        # DMAs land first in the prologue); the tail of the list is safe to
        # defer into the consuming pair's own body (late-uc es / tcx1 qsT).
        specs = []
        for hh in range(2):
            for jc in range(JC):
                specs.append(("q", (hh, jc, 0)))
        for uc in range(3):
            specs.append(("e", (uc,)))
        for hh in range(2):
            for jc in range(JC):
                specs.append(("q", (hh, jc, 1)))
        for uc in range(3, TB):
            specs.append(("e", (uc,)))
        thunks = []
        for i, (kind, args) in enumerate(specs):
            pool, tag = pool_cycle[i % len(pool_cycle)]
            alt = base_alt or (alt_from is not None and i >= alt_from)
            if kind == "q":
                thunks.append(
                    lambda a=args, pool=pool, tag=tag, alt=alt:
                    qsT_group(*a, pool, tag, alt))
            else:
                thunks.append(
                    lambda a=args, pool=pool, tag=tag, alt=alt:
                    es_group(*a, pool, tag, alt))
        return qsT2, es2, thunks

    def body(p, qsT2, es2, nthunks, drain, drate=2, stripes=False):
        # stripes=True (last pair): r_off(tb) = qs[tb] @ S[tb-1] + explicit
        # Omega[tb, tb-1] stripe, so each Gsum snapshot is consumed TWO
        # iterations after it is taken — the PSUM->SBUF cast latency can
        # never stall the PE even with no next-pair prep to hide it behind.
        pg2 = pgp.tile([P, JC, 512], F32, tag="pg", name=f"pg{p}")
        gs_prev = None
        gs_prev2 = None
        gs = None
        pwd = None
        omd = None
        stro = None
        stro_prev = None
        rp = [None] * 4        # rp chunk tiles, one per 2-bt
        rp_started = [False] * 4
        for bt in range(TB + 1):
            if bt < TB:
                # [A] diag OmegaT block for both heads: [u, (h, t)]
                if bt % 2 == 0:
                    pwd = pdp.tile([P, 4, P], F32, tag="pwd", name="pwd")
                for jc in range(JC):
                    nc.tensor.matmul(
                        pwd[:, 2 * (bt % 2):2 * (bt % 2) + 2, :],
                        lhsT=sT_sb[:, bt // 4, jc, (bt % 4) * P:(bt % 4 + 1) * P],
                        rhs=qsT2[:, :, jc, bt * P:(bt + 1) * P],
                        start=(bt % 2 == 0 and jc == 0),
                        stop=(bt % 2 == 1 and jc == JC - 1),
                        skip_group_check=True,
                    )
                if stripes and bt >= 1:
                    # stripe OmegaT[u in bt-1, (h, t in bt)] (full block, no
                    # mask); consumed by [G] next iteration
                    pstro = pdp.tile([P, 4, P], F32, tag="pwd", name="pstro")
                    for jc in range(JC):
                        nc.tensor.matmul(
                            pstro[:, 0:2, :],
                            lhsT=sT_sb[:, (bt - 1) // 4, jc,
                                       ((bt - 1) % 4) * P:((bt - 1) % 4 + 1) * P],
                            rhs=qsT2[:, :, jc, bt * P:(bt + 1) * P],
                            start=(jc == 0), stop=(jc == JC - 1),
                            skip_group_check=True,
                        )
                    stro = snap.tile([P, 2, P], BF16, tag="stro")
                    nc.scalar.copy(out=stro, in_=pstro[:, 0:2, :])
            # [G] r_off(bt-1) = qs[bt-1] @ Gsum[bt-1] via last iter's snapshot.
            # MUST be emitted before [B]: [G]'s wait on the gs copy transitively
            # (via the PE FIFO) keeps this iteration's pg2-accumulating matmuls
            # from racing ahead of last iteration's snapshot read.
            if bt >= 2:
                tb = bt - 1
                k = tb // 2
                if rp[k] is None:
                    rp[k] = prp.tile([P, 2, NF], F32, tag="rp", name=f"rp{k}")
                if stripes:
                    for hh in range(2):
                        nc.tensor.matmul(
                            rp[k][:, tb % 2, :],
                            lhsT=stro_prev[:, hh, :],
                            rhs=es2[:, tb - 1, hh * NF:(hh + 1) * NF],
                            start=(not rp_started[k]),
                            stop=(tb == 1 and hh == 1),
                            skip_group_check=True,
                        )
                        rp_started[k] = True
                if not stripes or tb >= 2:
                    gsrc = gs_prev2 if stripes else gs_prev
                    for hh in range(2):
                        for jc in range(JC):
                            nc.tensor.matmul(
                                rp[k][:, tb % 2, :],
                                lhsT=qsT2[:, hh, jc, tb * P:(tb + 1) * P],
                                rhs=gsrc[:, jc, hh * NF:(hh + 1) * NF],
                                start=(not rp_started[k]),
                                stop=(tb % 2 == 1 and hh == 1 and jc == JC - 1),
                                skip_group_check=True,
                            )
                            rp_started[k] = True
            if bt < TB:
                # [B] Gsum prefix accumulation + [C] snapshot. Under stripes
                # S[7] is never consumed: skip the last accumulate+snapshot.
                if bt >= 1 and not (stripes and bt == TB - 1):
                    uc = bt - 1
                    if stripes and gs_prev is not None:
                        # PE-FIFO guard: orders this iteration's accumulate
                        # after last iteration's snapshot read (Tile emits no
                        # WAR edge for mid-group PSUM reads).
                        nc.tensor.ldweights(weights=gs_prev[:, 0, 0:P])
                    for jc in range(JC):
                        nc.tensor.matmul(
                            pg2[:, jc, :],
                            lhsT=s_sb[:, uc, jc * P:(jc + 1) * P],
                            rhs=es2[:, uc, :],
                            # pg2 spans two banks (one per jc): each bank's
                            # first matmul needs start=True to clear its own
                            # has_written bits (start only clears ONE bank).
                            start=(bt == 1),
                            stop=(bt == (TB - 2 if stripes else TB - 1)
                                  and jc == JC - 1),
                            skip_group_check=True,
                        )
                    gs = snap.tile([P, JC, 512], BF16, tag="gs")
                    nc.vector.tensor_copy(out=gs[:, 0], in_=pg2[:, 0])
                    nc.scalar.copy(out=gs[:, 1], in_=pg2[:, 1])
            # [H] chunk complete -> accumulate into SBUF r, drain if last.
            # Placed after [C] so the gs copy leads the DVE queue.
            if bt >= 2 and (bt - 1) % 2 == 1:
                k = (bt - 1) // 2
                sl = r_sb[:, 2 * k:2 * k + 2, :]
                if p == 0:
                    nc.vector.tensor_copy(out=sl, in_=rp[k])
                else:
                    nc.vector.tensor_add(out=sl, in0=rp[k], in1=sl)
                if drain:
                    nc.sync.dma_start(out=out_d[:, 2 * k:2 * k + 2, :], in_=sl)
            if bt < TB:
                # [D] prep groups, interleaved as PE filler
                for _ in range(drate):
                    if nthunks:
                        nthunks.pop(0)()
                # [E]+[F] mask the diag pair, then its r contribution
                if bt % 2 == 1:
                    omd = snap.tile([P, 4, P], BF16, tag="omd")
                    nc.vector.tensor_mul(omd, pwd, mask4)
                    for b2 in (bt - 1, bt):
                        k = b2 // 2
                        if rp[k] is None:
                            rp[k] = prp.tile([P, 2, NF], F32, tag="rp",
                                             name=f"rp{k}")
                        for hh in range(2):
                            nc.tensor.matmul(
                                rp[k][:, b2 % 2, :],
                                lhsT=omd[:, 2 * (b2 % 2) + hh, :],
                                rhs=es2[:, b2, hh * NF:(hh + 1) * NF],
                                start=(not rp_started[k]),
                                stop=False,
                                skip_group_check=True,
                            )
                            rp_started[k] = True
            gs_prev2 = gs_prev
            gs_prev = gs
            stro_prev = stro

    # pair-0 prep runs bare during the DMA ramp; rotate over all three
    # transient PSUM pools so evacuation latency never blocks the PE. The
    # deferred bulk DMAs are emitted between groups so the scalar/gpsimd
    # queues enqueue them only once the critical transfers are in flight.
    qsT2, es2, thunks = prep_groups(
        0, pool_cycle=[(pwp, "pw"), (pdp, "pwd"), (prp, "rp")])
    for th in thunks:
        th()
    carry = []   # pair-3 prep groups deferred into pair 3's own body as
    # PE filler (it has no next-pair prep to hide the gs-copy latency behind)
    for p in range(NPAIR):
        if p + 1 < NPAIR:
            nqsT2, nes2, nthunks = prep_groups(
                p + 1, alt_from=(7 if p + 1 == NPAIR - 1 else None))
            if p + 1 == NPAIR - 1:
                nthunks, carry = nthunks[:7], nthunks[7:]
        else:
            nqsT2, nes2, nthunks = None, None, carry
        body(p, qsT2, es2, nthunks, drain=(p == NPAIR - 1),
             drate=(1 if p == NPAIR - 2 else 2),
             stripes=(p == NPAIR - 1))
        for th in nthunks:   # any leftovers
            th()
        qsT2, es2 = nqsT2, nes2


def build():
    from contextlib import ExitStack

    nc = bacc.Bacc(
        "TRN2",
        target_bir_lowering=False,
        debug=False,
        enable_asserts=False,
        num_devices=NCORES,
    )
    s_d = nc.dram_tensor("s", [P, TB, NF], BF16, kind="ExternalInput").ap()
    sT_d = nc.dram_tensor("sT", [P, 2, JC, 512], BF16,
                          kind="ExternalInput").ap()
    Q_d = nc.dram_tensor("Q", [P, H * JC, NF], BF16, kind="ExternalInput").ap()
    ET_d = nc.dram_tensor("ET", [P, JC * H, NF], BF16,
                          kind="ExternalInput").ap()
    out_d = nc.dram_tensor("out", [P, TB, NF], F32, kind="ExternalOutput").ap()
    with tile.TileContext(nc) as tc:
        with ExitStack() as ctx:
            _emit(tc, nc, s_d, sT_d, Q_d, ET_d, out_d, ctx)
    nc.compile()
    return nc


_NC = None


def _get_nc():
    global _NC
    if _NC is None:
        _NC = build()
    return _NC


def _in_maps(s, Q, E):
    bf = ml_dtypes.bfloat16
    s = np.asarray(s, np.float32)
    Qf = np.asarray(Q, np.float32)
    Ef = np.asarray(E, np.float32)
    Qd = np.ascontiguousarray(
        Qf.reshape(H, JC, P, NF).transpose(2, 0, 1, 3).reshape(P, H * JC, NF)
    ).astype(bf)
    ETd = np.ascontiguousarray(
        Ef.transpose(2, 0, 1).reshape(JC, P, H, NF).transpose(1, 0, 2, 3)
        .reshape(P, JC * H, NF)
    ).astype(bf)
    maps = []
    for b in range(B):
        sb = s[b]
        sd = np.ascontiguousarray(
            sb.reshape(TB, P, NF).transpose(1, 0, 2)).astype(bf)
        sTd = np.ascontiguousarray(
            sb.T.reshape(JC, P, 2, 512).transpose(1, 2, 0, 3)).astype(bf)
        maps.append({"s": sd, "sT": sTd, "Q": Qd, "ET": ETd})
    return maps


def _unpack(res):
    return np.stack([
        np.ascontiguousarray(
            res.results[b]["out"].transpose(1, 0, 2).reshape(T, NF))
        for b in range(B)], axis=0)


def kernel(s, Q, E):
    nc = _get_nc()
    res = run_bass_kernel_spmd(
        nc, _in_maps(s, Q, E), core_ids=list(range(NCORES)))
    return _unpack(res)


def run_profiled(s, Q, E, tmpdir=None):
    nc = _get_nc()
    res = run_bass_kernel_spmd(
        nc, _in_maps(s, Q, E), core_ids=list(range(NCORES)),
        trace=True, tmpdir=tmpdir)
    return _unpack(res), res.exec_time_ns


# revision 27
# speedup vs baseline: 1.1325x; 1.0005x over previous
"""Trainium2 Bass kernel (v10) for nn_AttentionBlock — reassociated causal attention.

Reference (per batch b):
    qs[t,j]    = sum_i s[t,i] Q[h,i,j]
    Omega[t,u] = sum_j qs[t,j] s[u,j]       (causal: keep u <= t)
    es[u,i]    = sum_j E[h,i,j] s[u,j]
    r[t,i]     = sum_h sum_u Omega[t,u] es[u,i]

Reassociation: for full (below-diagonal) 128-token blocks,
    sum_{u in blk} Omega[t,u] es[u,i] = qs[t,:] @ (s[blk].T @ es[blk])
so r's off-diagonal part = qs[bt] @ Gsum(bt) with Gsum the PSUM-accumulated
prefix of G_uc = s[uc].T @ es[uc]; only diagonal 128x128 Omega blocks are
materialized, masked by a DVE multiply with a precomputed triangular mask.

v17 over v9: heads processed in PAIRS with concatenated rhs operands (es, G,
and diag-Omega matmuls run at N=512/N=256 over both heads — 422 vs 601 PE
instructions), r accumulated in SBUF f32 via per-chunk DVE adds (frees 4 PSUM
banks for deeper transient rotation and removes the end-of-kernel PSUM
drain), the Gsum snapshot is consumed one bt-iteration late (software
pipeline; the r_off matmuls are emitted BEFORE the G-accumulate so the PE
FIFO transitively orders each snapshot read before the next accumulate —
Tile does not emit that WAR edge itself), all DRAM tensors are pre-swizzled
on the host into [128, contiguous] layouts (12 large 2KB+/partition-line
DMAs at full HBM bandwidth), a burst of dummy matmuls at t=0 warms the PE
HAM clock gate during the input-DMA ramp, and the last head-pair (which has
no next-pair prep) gets 9 of its own prep groups deferred into its body as
PE filler PLUS a "stripe" formulation (r_off(tb) = qs[tb] @ S[tb-1] + an
explicit full Omega[tb,tb-1] block) that consumes each Gsum snapshot two
iterations after capture — the copy latency can no longer stall the PE and
the HAM clock gate stays at 8/8 through the whole tail (an explicit
ldweights on the snapshot guards the read-before-next-accumulate order).

Gotcha encoded here: a PSUM accumulation group spanning N banks needs
start=True on EACH bank's first matmul (start clears has_written for one
bank only).

Distribution: data-parallel over batch (8 batches = 8 cores, no collectives).
All matmuls bf16; f32 PSUM accumulation. ~99.6us vs 111us baseline; rel err
3.07e-3. Note: back-to-back benchmarking runs heat the part into a lower
power state (~+20% exec time); space measurement runs out.
"""

import numpy as np
import ml_dtypes

import concourse.bacc as bacc
import concourse.mybir as mybir
import concourse.tile as tile
from concourse.bass_utils import run_bass_kernel_spmd

B = 8      # batch (== number of cores)
T = 1024   # tokens
NF = 256   # feature dim n
H = 8      # heads
P = 128    # partitions
TB = T // P    # 8 token blocks
JC = NF // P   # 2 feature chunks
NPAIR = H // 2
NCORES = 8

F32 = mybir.dt.float32
BF16 = mybir.dt.bfloat16
IS_GE = mybir.AluOpType.is_ge


def _emit(tc, nc, s_d, sT_d, Q_d, ET_d, out_d, ctx):
    res = ctx.enter_context(tc.tile_pool(name="res", bufs=1))
    work = ctx.enter_context(tc.tile_pool(name="work", bufs=2))
    snap = ctx.enter_context(tc.tile_pool(name="snap", bufs=2))
    prp = ctx.enter_context(tc.tile_pool(name="prp", bufs=1, space="PSUM"))
    pgp = ctx.enter_context(tc.tile_pool(name="pgp", bufs=1, space="PSUM"))
    pwp = ctx.enter_context(tc.tile_pool(name="pwp", bufs=2, space="PSUM"))
    pdp = ctx.enter_context(tc.tile_pool(name="pdp", bufs=3, space="PSUM"))

    s_sb = res.tile([P, TB, NF], BF16)       # [u%128, uc, j]
    sT_sb = res.tile([P, 2, JC, 512], BF16)  # [j%128, tcx, jc, t']
    Q_sb = res.tile([P, H * JC, NF], BF16)   # [i%128, h*2+ic, j]
    ET_sb = res.tile([P, JC * H, NF], BF16)  # [j%128, jc*8+h, i]
    mask4 = res.tile([P, 4, P], BF16)        # [u, (bt%2, h), t]: 1 where u <= t
    warm = res.tile([P, 640], BF16)
    r_sb = res.tile([P, TB, NF], F32)        # final r accumulator (SBUF)

    # ---- input DMAs: [128, contiguous] slices. The critical first wave
    # (both sT halves, pair-0 Q and ET) rides four queues in parallel so the
    # whole prologue working set lands ~3us after issue; bulk follows.
    nc.sync.dma_start(out=sT_sb[:, 0], in_=sT_d[:, 0])
    nc.sync.dma_start(out=sT_sb[:, 1], in_=sT_d[:, 1])
    nc.scalar.dma_start(out=Q_sb[:, 0:4, :], in_=Q_d[:, 0:4, :])
    nc.gpsimd.dma_start(out=ET_sb[:, 0:2, :], in_=ET_d[:, 0:2, :])
    nc.gpsimd.dma_start(out=ET_sb[:, 8:10, :], in_=ET_d[:, 8:10, :])
    nc.sync.dma_start(out=s_sb, in_=s_d)
    nc.scalar.dma_start(out=Q_sb[:, 4:8, :], in_=Q_d[:, 4:8, :])
    nc.gpsimd.dma_start(out=ET_sb[:, 2:4, :], in_=ET_d[:, 2:4, :])
    nc.gpsimd.dma_start(out=ET_sb[:, 10:12, :], in_=ET_d[:, 10:12, :])
    nc.scalar.dma_start(out=Q_sb[:, 8:16, :], in_=Q_d[:, 8:16, :])
    nc.gpsimd.dma_start(out=ET_sb[:, 4:8, :], in_=ET_d[:, 4:8, :])
    nc.gpsimd.dma_start(out=ET_sb[:, 12:16, :], in_=ET_d[:, 12:16, :])

    # warm tile zeroed on the (idle) vector engine so warmup matmuls are not
    # queued behind gpsimd's DMA issues; mask setup stays on gpsimd.
    nc.vector.memset(warm, 0.0)
    nc.gpsimd.memset(mask4, 1.0)
    nc.gpsimd.affine_select(
        out=mask4, in_=mask4,
        pattern=[[0, 4], [1, P]],
        compare_op=IS_GE,   # keep 1.0 where t - u >= 0, else 0
        fill=0.0, base=0, channel_multiplier=-1,
    )

    # HAM warmup: dummy matmuls on a zeroed tile while input DMAs run, so the
    # PE clock gate is at 8/8 by the time real matmuls start.
    pwarm = pdp.tile([P, 512], F32, tag="pwd", name="pwarm")
    for _ in range(9):
        nc.tensor.matmul(pwarm, lhsT=warm[:, 0:128], rhs=warm[:, 128:640],
                         start=True, stop=True, skip_group_check=True)

    # Prep evacuations alternate VectorE/ScalarE.
    movers = [nc.vector.tensor_copy, nc.scalar.copy]
    mv = [0]

    def mover(out, in_, alt=True):
        movers[mv[0] % 2](out=out, in_=in_)
        mv[0] += 1

    # ---- per-pair prep: qsT for both heads and pair-concatenated es
    def prep_groups(p, pool_cycle=None, alt_from=None):
        h0 = 2 * p
        qsT2 = work.tile([P, 2, JC, T], BF16, tag="qsT", name=f"qsT{p}")
        es2 = work.tile([P, TB, 2 * NF], BF16, tag="es", name=f"es{p}")
        base_alt = pool_cycle is not None
        pool_cycle = pool_cycle or [(pwp, "pw")]

        def qsT_group(hh, jc, tcx, pool, tag, alt):
            pw = pool.tile([P, 512], F32, tag=tag, name="pwq")
            for ic in range(JC):
                nc.tensor.matmul(
                    pw,
                    lhsT=Q_sb[:, (h0 + hh) * 2 + ic, jc * P:(jc + 1) * P],
                    rhs=sT_sb[:, tcx, ic, :],
                    start=(ic == 0), stop=(ic == JC - 1),
                    skip_group_check=True,
                )
            mover(qsT2[:, hh, jc, tcx * 512:(tcx + 1) * 512], pw, alt=alt)

        def es_group(uc, pool, tag, alt):
            pw = pool.tile([P, 512], F32, tag=tag, name="pwe")
            for jc in range(JC):
                nc.tensor.matmul(
                    pw,
                    lhsT=sT_sb[:, uc // 4, jc, (uc % 4) * P:(uc % 4 + 1) * P],
                    rhs=ET_sb[:, jc * H + h0:jc * H + h0 + 2, :],
                    start=(jc == 0), stop=(jc == JC - 1),
                    skip_group_check=True,
                )
            mover(es2[:, uc, :], pw, alt=alt)

        # Order groups progressively: tcx0-qsT and low-uc es first (their
        # DMAs land first in the prologue); the tail of the list is safe to
        # defer into the consuming pair's own body (late-uc es / tcx1 qsT).
        specs = []
        for hh in range(2):
            for jc in range(JC):
                specs.append(("q", (hh, jc, 0)))
        for uc in range(3):
            specs.append(("e", (uc,)))
        for hh in range(2):
            for jc in range(JC):
                specs.append(("q", (hh, jc, 1)))
        for uc in range(3, TB):
            specs.append(("e", (uc,)))
        thunks = []
        for i, (kind, args) in enumerate(specs):
            pool, tag = pool_cycle[i % len(pool_cycle)]
            alt = base_alt or (alt_from is not None and i >= alt_from)
            if kind == "q":
                thunks.append(
                    lambda a=args, pool=pool, tag=tag, alt=alt:
                    qsT_group(*a, pool, tag, alt))
            else:
                thunks.append(
                    lambda a=args, pool=pool, tag=tag, alt=alt:
                    es_group(*a, pool, tag, alt))
        return qsT2, es2, thunks

    def body(p, qsT2, es2, nthunks, drain, drate=2, stripes=False):
        # stripes=True (last pair): r_off(tb) = qs[tb] @ S[tb-1] + explicit
        # Omega[tb, tb-1] stripe, so each Gsum snapshot is consumed TWO
        # iterations after it is taken — the PSUM->SBUF cast latency can
        # never stall the PE even with no next-pair prep to hide it behind.
        pg2 = pgp.tile([P, JC, 512], F32, tag="pg", name=f"pg{p}")
        gs_prev = None
        gs_prev2 = None
        gs = None
        pwd = None
        omd = None
        stro = None
        stro_prev = None
        rp = [None] * 4        # rp chunk tiles, one per 2-bt
        rp_started = [False] * 4
        for bt in range(TB + 1):
            if bt < TB:
                # [A] diag OmegaT block for both heads: [u, (h, t)]
                if bt % 2 == 0:
                    pwd = pdp.tile([P, 4, P], F32, tag="pwd", name="pwd")
                for jc in range(JC):
                    nc.tensor.matmul(
                        pwd[:, 2 * (bt % 2):2 * (bt % 2) + 2, :],
                        lhsT=sT_sb[:, bt // 4, jc, (bt % 4) * P:(bt % 4 + 1) * P],
                        rhs=qsT2[:, :, jc, bt * P:(bt + 1) * P],
                        start=(bt % 2 == 0 and jc == 0),
                        stop=(bt % 2 == 1 and jc == JC - 1),
                        skip_group_check=True,
                    )
                if stripes and bt >= 1:
                    # stripe OmegaT[u in bt-1, (h, t in bt)] (full block, no
                    # mask); consumed by [G] next iteration
                    pstro = pdp.tile([P, 4, P], F32, tag="pwd", name="pstro")
                    for jc in range(JC):
                        nc.tensor.matmul(
                            pstro[:, 0:2, :],
                            lhsT=sT_sb[:, (bt - 1) // 4, jc,
                                       ((bt - 1) % 4) * P:((bt - 1) % 4 + 1) * P],
                            rhs=qsT2[:, :, jc, bt * P:(bt + 1) * P],
                            start=(jc == 0), stop=(jc == JC - 1),
                            skip_group_check=True,
                        )
                    stro = snap.tile([P, 2, P], BF16, tag="stro")
                    nc.scalar.copy(out=stro, in_=pstro[:, 0:2, :])
            # [G] r_off(bt-1) = qs[bt-1] @ Gsum[bt-1] via last iter's snapshot.
            # MUST be emitted before [B]: [G]'s wait on the gs copy transitively
            # (via the PE FIFO) keeps this iteration's pg2-accumulating matmuls
            # from racing ahead of last iteration's snapshot read.
            if bt >= 2:
                tb = bt - 1
                k = tb // 2
                if rp[k] is None:
                    rp[k] = prp.tile([P, 2, NF], F32, tag="rp", name=f"rp{k}")
                if stripes:
                    for hh in range(2):
                        nc.tensor.matmul(
                            rp[k][:, tb % 2, :],
                            lhsT=stro_prev[:, hh, :],
                            rhs=es2[:, tb - 1, hh * NF:(hh + 1) * NF],
                            start=(not rp_started[k]),
                            stop=(tb == 1 and hh == 1),
                            skip_group_check=True,
                        )
                        rp_started[k] = True
                if not stripes or tb >= 2:
                    gsrc = gs_prev2 if stripes else gs_prev
                    for hh in range(2):
                        for jc in range(JC):
                            nc.tensor.matmul(
                                rp[k][:, tb % 2, :],
                                lhsT=qsT2[:, hh, jc, tb * P:(tb + 1) * P],
                                rhs=gsrc[:, jc, hh * NF:(hh + 1) * NF],
                                start=(not rp_started[k]),
                                stop=(tb % 2 == 1 and hh == 1 and jc == JC - 1),
                                skip_group_check=True,
                            )
                            rp_started[k] = True
            if bt < TB:
                # [B] Gsum prefix accumulation + [C] snapshot. Under stripes
                # S[7] is never consumed: skip the last accumulate+snapshot.
                if bt >= 1 and not (stripes and bt == TB - 1):
                    uc = bt - 1
                    if stripes and gs_prev is not None:
                        # PE-FIFO guard: orders this iteration's accumulate
                        # after last iteration's snapshot read (Tile emits no
                        # WAR edge for mid-group PSUM reads).
                        nc.tensor.ldweights(weights=gs_prev[:, 0, 0:P])
                    for jc in range(JC):
                        nc.tensor.matmul(
                            pg2[:, jc, :],
                            lhsT=s_sb[:, uc, jc * P:(jc + 1) * P],
                            rhs=es2[:, uc, :],
                            # pg2 spans two banks (one per jc): each bank's
                            # first matmul needs start=True to clear its own
                            # has_written bits (start only clears ONE bank).
                            start=(bt == 1),
                            stop=(bt == (TB - 2 if stripes else TB - 1)
                                  and jc == JC - 1),
                            skip_group_check=True,
                        )
                    gs = snap.tile([P, JC, 512], BF16, tag="gs")
                    nc.vector.tensor_copy(out=gs[:, 0], in_=pg2[:, 0])
                    nc.scalar.copy(out=gs[:, 1], in_=pg2[:, 1])
            # [H] chunk complete -> accumulate into SBUF r, drain if last.
            # Pairs 1-2 route the add via ScalarE-evac + GpSimd (keeps the
            # DVE queue clear so gs casts land promptly); the last pair keeps
            # the single DVE add so the drain chain stays short.
            if bt >= 2 and (bt - 1) % 2 == 1:
                k = (bt - 1) // 2
                sl = r_sb[:, 2 * k:2 * k + 2, :]
                if p == 0:
                    nc.scalar.copy(out=sl, in_=rp[k])
                elif not drain:
                    rps = snap.tile([P, 2, NF], F32, tag="rps")
                    nc.scalar.copy(out=rps, in_=rp[k])
                    nc.gpsimd.tensor_add(out=sl, in0=rps, in1=sl)
                else:
                    nc.vector.tensor_add(out=sl, in0=rp[k], in1=sl)
                if drain:
                    nc.sync.dma_start(out=out_d[:, 2 * k:2 * k + 2, :], in_=sl)
            if bt < TB:
                # [D] prep groups, interleaved as PE filler
                for _ in range(drate):
                    if nthunks:
                        nthunks.pop(0)()
                # [E]+[F] mask the diag pair, then its r contribution
                if bt % 2 == 1:
                    omd = snap.tile([P, 4, P], BF16, tag="omd")
                    nc.vector.tensor_mul(omd, pwd, mask4)
                    for b2 in (bt - 1, bt):
                        k = b2 // 2
                        if rp[k] is None:
                            rp[k] = prp.tile([P, 2, NF], F32, tag="rp",
                                             name=f"rp{k}")
                        for hh in range(2):
                            nc.tensor.matmul(
                                rp[k][:, b2 % 2, :],
                                lhsT=omd[:, 2 * (b2 % 2) + hh, :],
                                rhs=es2[:, b2, hh * NF:(hh + 1) * NF],
                                start=(not rp_started[k]),
                                stop=False,
                                skip_group_check=True,
                            )
                            rp_started[k] = True
            gs_prev2 = gs_prev
            gs_prev = gs
            stro_prev = stro

    # pair-0 prep runs bare during the DMA ramp; rotate over all three
    # transient PSUM pools so evacuation latency never blocks the PE. The
    # deferred bulk DMAs are emitted between groups so the scalar/gpsimd
    # queues enqueue them only once the critical transfers are in flight.
    qsT2, es2, thunks = prep_groups(
        0, pool_cycle=[(pwp, "pw"), (pdp, "pwd"), (prp, "rp")])
    for th in thunks:
        th()
    carry = []   # pair-3 prep groups deferred into pair 3's own body as
    # PE filler (it has no next-pair prep to hide the gs-copy latency behind)
    for p in range(NPAIR):
        if p + 1 < NPAIR:
            nqsT2, nes2, nthunks = prep_groups(
                p + 1, alt_from=(7 if p + 1 == NPAIR - 1 else None))
            if p + 1 == NPAIR - 1:
                nthunks, carry = nthunks[:7], nthunks[7:]
        else:
            nqsT2, nes2, nthunks = None, None, carry
        body(p, qsT2, es2, nthunks, drain=(p == NPAIR - 1),
             drate=(1 if p == NPAIR - 2 else 2),
             stripes=(p == NPAIR - 1))
        for th in nthunks:   # any leftovers
            th()
        qsT2, es2 = nqsT2, nes2


def build():
    from contextlib import ExitStack

    nc = bacc.Bacc(
        "TRN2",
        target_bir_lowering=False,
        debug=False,
        enable_asserts=False,
        num_devices=NCORES,
    )
    s_d = nc.dram_tensor("s", [P, TB, NF], BF16, kind="ExternalInput").ap()
    sT_d = nc.dram_tensor("sT", [P, 2, JC, 512], BF16,
                          kind="ExternalInput").ap()
    Q_d = nc.dram_tensor("Q", [P, H * JC, NF], BF16, kind="ExternalInput").ap()
    ET_d = nc.dram_tensor("ET", [P, JC * H, NF], BF16,
                          kind="ExternalInput").ap()
    out_d = nc.dram_tensor("out", [P, TB, NF], F32, kind="ExternalOutput").ap()
    with tile.TileContext(nc) as tc:
        with ExitStack() as ctx:
            _emit(tc, nc, s_d, sT_d, Q_d, ET_d, out_d, ctx)
    nc.compile()
    return nc


_NC = None


def _get_nc():
    global _NC
    if _NC is None:
        _NC = build()
    return _NC


def _in_maps(s, Q, E):
    bf = ml_dtypes.bfloat16
    s = np.asarray(s, np.float32)
    Qf = np.asarray(Q, np.float32)
    Ef = np.asarray(E, np.float32)
    Qd = np.ascontiguousarray(
        Qf.reshape(H, JC, P, NF).transpose(2, 0, 1, 3).reshape(P, H * JC, NF)
    ).astype(bf)
    ETd = np.ascontiguousarray(
        Ef.transpose(2, 0, 1).reshape(JC, P, H, NF).transpose(1, 0, 2, 3)
        .reshape(P, JC * H, NF)
    ).astype(bf)
    maps = []
    for b in range(B):
        sb = s[b]
        sd = np.ascontiguousarray(
            sb.reshape(TB, P, NF).transpose(1, 0, 2)).astype(bf)
        sTd = np.ascontiguousarray(
            sb.T.reshape(JC, P, 2, 512).transpose(1, 2, 0, 3)).astype(bf)
        maps.append({"s": sd, "sT": sTd, "Q": Qd, "ET": ETd})
    return maps


def _unpack(res):
    return np.stack([
        np.ascontiguousarray(
            res.results[b]["out"].transpose(1, 0, 2).reshape(T, NF))
        for b in range(B)], axis=0)


def kernel(s, Q, E):
    nc = _get_nc()
    res = run_bass_kernel_spmd(
        nc, _in_maps(s, Q, E), core_ids=list(range(NCORES)))
    return _unpack(res)


def run_profiled(s, Q, E, tmpdir=None):
    nc = _get_nc()
    res = run_bass_kernel_spmd(
        nc, _in_maps(s, Q, E), core_ids=list(range(NCORES)),
        trace=True, tmpdir=tmpdir)
    return _unpack(res), res.exec_time_ns


# revision 30
# speedup vs baseline: 1.1417x; 1.0081x over previous
"""Trainium2 Bass kernel (v10) for nn_AttentionBlock — reassociated causal attention.

Reference (per batch b):
    qs[t,j]    = sum_i s[t,i] Q[h,i,j]
    Omega[t,u] = sum_j qs[t,j] s[u,j]       (causal: keep u <= t)
    es[u,i]    = sum_j E[h,i,j] s[u,j]
    r[t,i]     = sum_h sum_u Omega[t,u] es[u,i]

Reassociation: for full (below-diagonal) 128-token blocks,
    sum_{u in blk} Omega[t,u] es[u,i] = qs[t,:] @ (s[blk].T @ es[blk])
so r's off-diagonal part = qs[bt] @ Gsum(bt) with Gsum the PSUM-accumulated
prefix of G_uc = s[uc].T @ es[uc]; only diagonal 128x128 Omega blocks are
materialized, masked by a DVE multiply with a precomputed triangular mask.

v18 over v9: heads processed in PAIRS with concatenated rhs operands (es, G,
and diag-Omega matmuls run at N=512/N=256 over both heads — 422 vs 601 PE
instructions), r accumulated in SBUF f32 via per-chunk DVE adds (frees 4 PSUM
banks for deeper transient rotation and removes the end-of-kernel PSUM
drain; pairs 1-2 route the per-chunk add ScalarE->SBUF then sum on the
otherwise-idle GpSimd so the DVE queue never delays a Gsum cast; the last
pair keeps a single DVE add so the output-drain chain stays short), the
Gsum snapshot is consumed one bt-iteration late (software
pipeline; the r_off matmuls are emitted BEFORE the G-accumulate so the PE
FIFO transitively orders each snapshot read before the next accumulate —
Tile does not emit that WAR edge itself), all DRAM tensors are pre-swizzled
on the host into [128, contiguous] layouts (12 large 2KB+/partition-line
DMAs at full HBM bandwidth), a burst of dummy matmuls at t=0 warms the PE
HAM clock gate during the input-DMA ramp, and the last head-pair (which has
no next-pair prep) gets 9 of its own prep groups deferred into its body as
PE filler PLUS a "stripe" formulation (r_off(tb) = qs[tb] @ S[tb-1] + an
explicit full Omega[tb,tb-1] block) that consumes each Gsum snapshot two
iterations after capture — the copy latency can no longer stall the PE and
the HAM clock gate stays at 8/8 through the whole tail (an explicit
ldweights on the snapshot guards the read-before-next-accumulate order).

Gotcha encoded here: a PSUM accumulation group spanning N banks needs
start=True on EACH bank's first matmul (start clears has_written for one
bank only).

Distribution: data-parallel over batch (8 batches = 8 cores, no collectives).
All matmuls bf16; f32 PSUM accumulation. ~99.1-99.5us vs 111us baseline; rel err
3.07e-3. Note: back-to-back benchmarking runs heat the part into a lower
power state (~+20% exec time); space measurement runs out.
"""

import numpy as np
import ml_dtypes

import concourse.bacc as bacc
import concourse.mybir as mybir
import concourse.tile as tile
from concourse.bass_utils import run_bass_kernel_spmd

B = 8      # batch (== number of cores)
T = 1024   # tokens
NF = 256   # feature dim n
H = 8      # heads
P = 128    # partitions
TB = T // P    # 8 token blocks
JC = NF // P   # 2 feature chunks
NPAIR = H // 2
NCORES = 8

F32 = mybir.dt.float32
BF16 = mybir.dt.bfloat16
IS_GE = mybir.AluOpType.is_ge


def _emit(tc, nc, s_d, sT_d, Q_d, ET_d, out_d, ctx):
    res = ctx.enter_context(tc.tile_pool(name="res", bufs=1))
    work = ctx.enter_context(tc.tile_pool(name="work", bufs=2))
    snap = ctx.enter_context(tc.tile_pool(name="snap", bufs=2))
    prp = ctx.enter_context(tc.tile_pool(name="prp", bufs=1, space="PSUM"))
    pgp = ctx.enter_context(tc.tile_pool(name="pgp", bufs=1, space="PSUM"))
    pwp = ctx.enter_context(tc.tile_pool(name="pwp", bufs=2, space="PSUM"))
    pdp = ctx.enter_context(tc.tile_pool(name="pdp", bufs=3, space="PSUM"))

    s_sb = res.tile([P, TB, NF], BF16)       # [u%128, uc, j]
    sT_sb = res.tile([P, 2, JC, 512], BF16)  # [j%128, tcx, jc, t']
    Q_sb = res.tile([P, H * JC, NF], BF16)   # [i%128, h*2+ic, j]
    ET_sb = res.tile([P, JC * H, NF], BF16)  # [j%128, jc*8+h, i]
    mask4 = res.tile([P, 4, P], BF16)        # [u, (bt%2, h), t]: 1 where u <= t
    warm = res.tile([P, 640], BF16)
    r_sb = res.tile([P, TB, NF], F32)        # final r accumulator (SBUF)

    # ---- input DMAs: [128, contiguous] slices. The critical first wave
    # (both sT halves, pair-0 Q and ET) rides four queues in parallel so the
    # whole prologue working set lands ~3us after issue; bulk follows.
    nc.sync.dma_start(out=sT_sb[:, 0], in_=sT_d[:, 0])
    nc.sync.dma_start(out=sT_sb[:, 1], in_=sT_d[:, 1])
    nc.scalar.dma_start(out=Q_sb[:, 0:4, :], in_=Q_d[:, 0:4, :])
    nc.gpsimd.dma_start(out=ET_sb[:, 0:2, :], in_=ET_d[:, 0:2, :])
    nc.gpsimd.dma_start(out=ET_sb[:, 8:10, :], in_=ET_d[:, 8:10, :])
    nc.sync.dma_start(out=s_sb, in_=s_d)
    nc.scalar.dma_start(out=Q_sb[:, 4:8, :], in_=Q_d[:, 4:8, :])
    nc.gpsimd.dma_start(out=ET_sb[:, 2:4, :], in_=ET_d[:, 2:4, :])
    nc.gpsimd.dma_start(out=ET_sb[:, 10:12, :], in_=ET_d[:, 10:12, :])
    nc.scalar.dma_start(out=Q_sb[:, 8:16, :], in_=Q_d[:, 8:16, :])
    nc.gpsimd.dma_start(out=ET_sb[:, 4:8, :], in_=ET_d[:, 4:8, :])
    nc.gpsimd.dma_start(out=ET_sb[:, 12:16, :], in_=ET_d[:, 12:16, :])

    # warm tile zeroed on the (idle) vector engine so warmup matmuls are not
    # queued behind gpsimd's DMA issues; mask setup stays on gpsimd.
    nc.vector.memset(warm, 0.0)
    nc.gpsimd.memset(mask4, 1.0)
    nc.gpsimd.affine_select(
        out=mask4, in_=mask4,
        pattern=[[0, 4], [1, P]],
        compare_op=IS_GE,   # keep 1.0 where t - u >= 0, else 0
        fill=0.0, base=0, channel_multiplier=-1,
    )

    # HAM warmup: dummy matmuls on a zeroed tile while input DMAs run, so the
    # PE clock gate is at 8/8 by the time real matmuls start.
    pwarm = pdp.tile([P, 512], F32, tag="pwd", name="pwarm")
    for _ in range(9):
        nc.tensor.matmul(pwarm, lhsT=warm[:, 0:128], rhs=warm[:, 128:640],
                         start=True, stop=True, skip_group_check=True)

    # Prep evacuations alternate VectorE/ScalarE.
    movers = [nc.vector.tensor_copy, nc.scalar.copy]
    mv = [0]

    def mover(out, in_, alt=True):
        movers[mv[0] % 2](out=out, in_=in_)
        mv[0] += 1

    # ---- per-pair prep: qsT for both heads and pair-concatenated es
    def prep_groups(p, pool_cycle=None, alt_from=None):
        h0 = 2 * p
        qsT2 = work.tile([P, 2, JC, T], BF16, tag="qsT", name=f"qsT{p}")
        es2 = work.tile([P, TB, 2 * NF], BF16, tag="es", name=f"es{p}")
        base_alt = pool_cycle is not None
        pool_cycle = pool_cycle or [(pwp, "pw")]

        def qsT_group(hh, jc, tcx, pool, tag, alt):
            pw = pool.tile([P, 512], F32, tag=tag, name="pwq")
            for ic in range(JC):
                nc.tensor.matmul(
                    pw,
                    lhsT=Q_sb[:, (h0 + hh) * 2 + ic, jc * P:(jc + 1) * P],
                    rhs=sT_sb[:, tcx, ic, :],
                    start=(ic == 0), stop=(ic == JC - 1),
                    skip_group_check=True,
                )
            mover(qsT2[:, hh, jc, tcx * 512:(tcx + 1) * 512], pw, alt=alt)

        def es_group(uc, pool, tag, alt):
            pw = pool.tile([P, 512], F32, tag=tag, name="pwe")
            for jc in range(JC):
                nc.tensor.matmul(
                    pw,
                    lhsT=sT_sb[:, uc // 4, jc, (uc % 4) * P:(uc % 4 + 1) * P],
                    rhs=ET_sb[:, jc * H + h0:jc * H + h0 + 2, :],
                    start=(jc == 0), stop=(jc == JC - 1),
                    skip_group_check=True,
                )
            mover(es2[:, uc, :], pw, alt=alt)

        # Order groups progressively: tcx0-qsT and low-uc es first (their
        # DMAs land first in the prologue); the tail of the list is safe to
        # defer into the consuming pair's own body (late-uc es / tcx1 qsT).
        specs = []
        for hh in range(2):
            for jc in range(JC):
                specs.append(("q", (hh, jc, 0)))
        for uc in range(3):
            specs.append(("e", (uc,)))
        for hh in range(2):
            for jc in range(JC):
                specs.append(("q", (hh, jc, 1)))
        for uc in range(3, TB):
            specs.append(("e", (uc,)))
        thunks = []
        for i, (kind, args) in enumerate(specs):
            pool, tag = pool_cycle[i % len(pool_cycle)]
            alt = base_alt or (alt_from is not None and i >= alt_from)
            if kind == "q":
                thunks.append(
                    lambda a=args, pool=pool, tag=tag, alt=alt:
                    qsT_group(*a, pool, tag, alt))
            else:
                thunks.append(
                    lambda a=args, pool=pool, tag=tag, alt=alt:
                    es_group(*a, pool, tag, alt))
        return qsT2, es2, thunks

    def body(p, qsT2, es2, nthunks, drain, drate=2, stripes=False):
        # stripes=True (last pair): r_off(tb) = qs[tb] @ S[tb-1] + explicit
        # Omega[tb, tb-1] stripe, so each Gsum snapshot is consumed TWO
        # iterations after it is taken — the PSUM->SBUF cast latency can
        # never stall the PE even with no next-pair prep to hide it behind.
        pg2 = pgp.tile([P, JC, 512], F32, tag="pg", name=f"pg{p}")
        gs_prev = None
        gs_prev2 = None
        gs = None
        pwd = None
        omd = None
        stro = None
        stro_prev = None
        rp = [None] * 4        # rp chunk tiles, one per 2-bt
        rp_started = [False] * 4
        for bt in range(TB + 1):
            if bt < TB:
                # [A] diag OmegaT block for both heads: [u, (h, t)]
                if bt % 2 == 0:
                    pwd = pdp.tile([P, 4, P], F32, tag="pwd", name="pwd")
                for jc in range(JC):
                    nc.tensor.matmul(
                        pwd[:, 2 * (bt % 2):2 * (bt % 2) + 2, :],
                        lhsT=sT_sb[:, bt // 4, jc, (bt % 4) * P:(bt % 4 + 1) * P],
                        rhs=qsT2[:, :, jc, bt * P:(bt + 1) * P],
                        start=(bt % 2 == 0 and jc == 0),
                        stop=(bt % 2 == 1 and jc == JC - 1),
                        skip_group_check=True,
                    )
                if stripes and bt >= 1:
                    # stripe OmegaT[u in bt-1, (h, t in bt)] (full block, no
                    # mask); consumed by [G] next iteration
                    pstro = pdp.tile([P, 4, P], F32, tag="pwd", name="pstro")
                    for jc in range(JC):
                        nc.tensor.matmul(
                            pstro[:, 0:2, :],
                            lhsT=sT_sb[:, (bt - 1) // 4, jc,
                                       ((bt - 1) % 4) * P:((bt - 1) % 4 + 1) * P],
                            rhs=qsT2[:, :, jc, bt * P:(bt + 1) * P],
                            start=(jc == 0), stop=(jc == JC - 1),
                            skip_group_check=True,
                        )
                    stro = snap.tile([P, 2, P], BF16, tag="stro")
                    nc.scalar.copy(out=stro, in_=pstro[:, 0:2, :])
            # [G] r_off(bt-1) = qs[bt-1] @ Gsum[bt-1] via last iter's snapshot.
            # MUST be emitted before [B]: [G]'s wait on the gs copy transitively
            # (via the PE FIFO) keeps this iteration's pg2-accumulating matmuls
            # from racing ahead of last iteration's snapshot read.
            if bt >= 2:
                tb = bt - 1
                k = tb // 2
                if rp[k] is None:
                    rp[k] = prp.tile([P, 2, NF], F32, tag="rp", name=f"rp{k}")
                if stripes:
                    for hh in range(2):
                        nc.tensor.matmul(
                            rp[k][:, tb % 2, :],
                            lhsT=stro_prev[:, hh, :],
                            rhs=es2[:, tb - 1, hh * NF:(hh + 1) * NF],
                            start=(not rp_started[k]),
                            stop=(tb == 1 and hh == 1),
                            skip_group_check=True,
                        )
                        rp_started[k] = True
                if not stripes or tb >= 2:
                    gsrc = gs_prev2 if stripes else gs_prev
                    for hh in range(2):
                        for jc in range(JC):
                            nc.tensor.matmul(
                                rp[k][:, tb % 2, :],
                                lhsT=qsT2[:, hh, jc, tb * P:(tb + 1) * P],
                                rhs=gsrc[:, jc, hh * NF:(hh + 1) * NF],
                                start=(not rp_started[k]),
                                stop=(tb % 2 == 1 and hh == 1 and jc == JC - 1),
                                skip_group_check=True,
                            )
                            rp_started[k] = True
            if bt < TB:
                # [B] Gsum prefix accumulation + [C] snapshot. Under stripes
                # S[7] is never consumed: skip the last accumulate+snapshot.
                if bt >= 1 and not (stripes and bt == TB - 1):
                    uc = bt - 1
                    if stripes and gs_prev is not None:
                        # PE-FIFO guard: orders this iteration's accumulate
                        # after last iteration's snapshot read (Tile emits no
                        # WAR edge for mid-group PSUM reads).
                        nc.tensor.ldweights(weights=gs_prev[:, 0, 0:P])
                    for jc in range(JC):
                        nc.tensor.matmul(
                            pg2[:, jc, :],
                            lhsT=s_sb[:, uc, jc * P:(jc + 1) * P],
                            rhs=es2[:, uc, :],
                            # pg2 spans two banks (one per jc): each bank's
                            # first matmul needs start=True to clear its own
                            # has_written bits (start only clears ONE bank).
                            start=(bt == 1),
                            stop=(bt == (TB - 2 if stripes else TB - 1)
                                  and jc == JC - 1),
                            skip_group_check=True,
                        )
                    gs = snap.tile([P, JC, 512], BF16, tag="gs")
                    nc.vector.tensor_copy(out=gs[:, 0], in_=pg2[:, 0])
                    nc.scalar.copy(out=gs[:, 1], in_=pg2[:, 1])
            # [H] chunk complete -> accumulate into SBUF r, drain if last.
            # Pairs 1-2 route the add via ScalarE-evac + GpSimd (keeps the
            # DVE queue clear so gs casts land promptly); the last pair keeps
            # the single DVE add so the drain chain stays short.
            if bt >= 2 and (bt - 1) % 2 == 1:
                k = (bt - 1) // 2
                sl = r_sb[:, 2 * k:2 * k + 2, :]
                if p == 0:
                    nc.scalar.copy(out=sl, in_=rp[k])
                elif not drain:
                    rps = snap.tile([P, 2, NF], F32, tag="rps")
                    nc.scalar.copy(out=rps, in_=rp[k])
                    nc.gpsimd.tensor_add(out=sl, in0=rps, in1=sl)
                else:
                    nc.vector.tensor_add(out=sl, in0=rp[k], in1=sl)
                if drain:
                    nc.sync.dma_start(out=out_d[:, 2 * k:2 * k + 2, :], in_=sl)
            if bt < TB:
                # [D] prep groups, interleaved as PE filler
                for _ in range(drate):
                    if nthunks:
                        nthunks.pop(0)()
                # [E]+[F] mask the diag pair, then its r contribution
                if bt % 2 == 1:
                    omd = snap.tile([P, 4, P], BF16, tag="omd")
                    nc.vector.tensor_mul(omd, pwd, mask4)
                    for b2 in (bt - 1, bt):
                        k = b2 // 2
                        if rp[k] is None:
                            rp[k] = prp.tile([P, 2, NF], F32, tag="rp",
                                             name=f"rp{k}")
                        for hh in range(2):
                            nc.tensor.matmul(
                                rp[k][:, b2 % 2, :],
                                lhsT=omd[:, 2 * (b2 % 2) + hh, :],
                                rhs=es2[:, b2, hh * NF:(hh + 1) * NF],
                                start=(not rp_started[k]),
                                stop=False,
                                skip_group_check=True,
                            )
                            rp_started[k] = True
            gs_prev2 = gs_prev
            gs_prev = gs
            stro_prev = stro

    # pair-0 prep runs bare during the DMA ramp; rotate over all three
    # transient PSUM pools so evacuation latency never blocks the PE. The
    # deferred bulk DMAs are emitted between groups so the scalar/gpsimd
    # queues enqueue them only once the critical transfers are in flight.
    qsT2, es2, thunks = prep_groups(
        0, pool_cycle=[(pwp, "pw"), (pdp, "pwd"), (prp, "rp")])
    for th in thunks:
        th()
    carry = []   # pair-3 prep groups deferred into pair 3's own body as
    # PE filler (it has no next-pair prep to hide the gs-copy latency behind)
    for p in range(NPAIR):
        if p + 1 < NPAIR:
            nqsT2, nes2, nthunks = prep_groups(
                p + 1, alt_from=(7 if p + 1 == NPAIR - 1 else None))
            if p + 1 == NPAIR - 1:
                nthunks, carry = nthunks[:7], nthunks[7:]
        else:
            nqsT2, nes2, nthunks = None, None, carry
        body(p, qsT2, es2, nthunks, drain=(p == NPAIR - 1),
             drate=(1 if p == NPAIR - 2 else 2),
             stripes=(p == NPAIR - 1))
        for th in nthunks:   # any leftovers
            th()
        qsT2, es2 = nqsT2, nes2


def build():
    from contextlib import ExitStack

    nc = bacc.Bacc(
        "TRN2",
        target_bir_lowering=False,
        debug=False,
        enable_asserts=False,
        num_devices=NCORES,
    )
    s_d = nc.dram_tensor("s", [P, TB, NF], BF16, kind="ExternalInput").ap()
    sT_d = nc.dram_tensor("sT", [P, 2, JC, 512], BF16,
                          kind="ExternalInput").ap()
    Q_d = nc.dram_tensor("Q", [P, H * JC, NF], BF16, kind="ExternalInput").ap()
    ET_d = nc.dram_tensor("ET", [P, JC * H, NF], BF16,
                          kind="ExternalInput").ap()
    out_d = nc.dram_tensor("out", [P, TB, NF], F32, kind="ExternalOutput").ap()
    with tile.TileContext(nc) as tc:
        with ExitStack() as ctx:
            _emit(tc, nc, s_d, sT_d, Q_d, ET_d, out_d, ctx)
    nc.compile()
    return nc


_NC = None


def _get_nc():
    global _NC
    if _NC is None:
        _NC = build()
    return _NC


def _in_maps(s, Q, E):
    bf = ml_dtypes.bfloat16
    s = np.asarray(s, np.float32)
    Qf = np.asarray(Q, np.float32)
    Ef = np.asarray(E, np.float32)
    Qd = np.ascontiguousarray(
        Qf.reshape(H, JC, P, NF).transpose(2, 0, 1, 3).reshape(P, H * JC, NF)
    ).astype(bf)
    ETd = np.ascontiguousarray(
        Ef.transpose(2, 0, 1).reshape(JC, P, H, NF).transpose(1, 0, 2, 3)
        .reshape(P, JC * H, NF)
    ).astype(bf)
    maps = []
    for b in range(B):
        sb = s[b]
        sd = np.ascontiguousarray(
            sb.reshape(TB, P, NF).transpose(1, 0, 2)).astype(bf)
        sTd = np.ascontiguousarray(
            sb.T.reshape(JC, P, 2, 512).transpose(1, 2, 0, 3)).astype(bf)
        maps.append({"s": sd, "sT": sTd, "Q": Qd, "ET": ETd})
    return maps


def _unpack(res):
    return np.stack([
        np.ascontiguousarray(
            res.results[b]["out"].transpose(1, 0, 2).reshape(T, NF))
        for b in range(B)], axis=0)


def kernel(s, Q, E):
    nc = _get_nc()
    res = run_bass_kernel_spmd(
        nc, _in_maps(s, Q, E), core_ids=list(range(NCORES)))
    return _unpack(res)


def run_profiled(s, Q, E, tmpdir=None):
    nc = _get_nc()
    res = run_bass_kernel_spmd(
        nc, _in_maps(s, Q, E), core_ids=list(range(NCORES)),
        trace=True, tmpdir=tmpdir)
    return _unpack(res), res.exec_time_ns
